# revision 44
# baseline (speedup 1.0000x reference)
"""Differential attention kernel for 8 trn2 NeuronCores.

Sharding: (batch, head-group) over 8 cores. Core d handles batch b=d//4 and
heads [4*(d%4), 4*(d%4)+4).

Schedule (v2): the ACT engine's exp work (~2.1us per key tile) is the
per-pass pace-setter, so the kernel is organized to start it early and
never let the PE starve it:
  - head: project only chunk 0 (k, v, q) -- DMA-gated ~18us -- then enter
    the attention pass for (chunk 0, pair 0) immediately,
  - pass (0,0) stretches: k/v/q projections of chunks 1..3 are emitted as
    interludes between key-tile groups, allocating their PSUM accumulators
    from the score-tile pool (the only free banks mid-pass),
  - later passes get boundary bursts (q-proj of a later chunk or the
    out-projection of the previous chunk) in the window where the combine
    has freed the u-accumulator banks,
  - scores run in bf16 (q_t, kz tiles bf16; inputs xt/wqk/wv bf16) which
    fits everything in SBUF simultaneously; PV/out-proj stay f32r.
Numerics: bf16 q/k/scores measured at ~4e-3 rel max vs the f32 reference
(gate is 2e-2); exp without max-subtraction is safe (|logit| < ~3).

Per-pass internals follow the v1 baseline: zero-padded kz tiles keep the
score matmuls at K=128 (fp32r/bf16 K=64 with alternating row groups is ~5x
slower), scores land transposed [key, query] so the mask is an ACT bias and
PV needs no transposes, v carries a ones column so softmax denominators
ride row 64 of the u accumulator, and the combine uses
reciprocal_approx_fast plus a DRAM round-trip partition broadcast.
"""
import numpy as np

B, S, D, H = 2, 2048, 1024, 16
DH = D // H          # 64
SCALE = DH ** -0.5   # 0.125
NCORES = 8
HG = 4               # heads per device
KT = D // 128        # 8 contraction tiles over D
MT = D // 128        # 8 output col-tiles of qk projection (q1,q2,k1,k2)
NCH = S // 512       # 4 query chunks
JT = S // 128        # 16 key tiles

_BUILD_CACHE = {}


def _build(lam: float):
    from contextlib import ExitStack
    import concourse.mybir as mybir
    import concourse.tile as tile
    from concourse import bacc

    f32 = mybir.dt.float32
    f32r = mybir.dt.float32r
    bf16 = mybir.dt.bfloat16
    Exp = mybir.ActivationFunctionType.Exp
    mult = mybir.AluOpType.mult
    add = mybir.AluOpType.add

    nc = bacc.Bacc("TRN2", target_bir_lowering=False, debug=False,
                   num_devices=NCORES)

    xt_d = nc.dram_tensor("xt", [D, S], bf16, kind="ExternalInput").ap()
    wqk_d = nc.dram_tensor("wqk", [D, D], bf16, kind="ExternalInput").ap()
    wv_d = nc.dram_tensor("wv", [D, HG * DH], bf16, kind="ExternalInput").ap()
    wo_d = nc.dram_tensor("wo", [HG * DH, D], f32r, kind="ExternalInput").ap()
    bqk_d = nc.dram_tensor("bqk", [128, MT], f32, kind="ExternalInput").ap()
    maskb_d = nc.dram_tensor("maskb", [128, JT], f32,
                             kind="ExternalInput").ap()
    out_d = nc.dram_tensor("outT", [D, S], f32, kind="ExternalOutput").ap()

    with tile.TileContext(nc) as tc, ExitStack() as ctx:
        consts = ctx.enter_context(tc.tile_pool(name="consts", bufs=1))
        w_pool = ctx.enter_context(tc.tile_pool(name="wp", bufs=1))
        qk_pool = ctx.enter_context(tc.tile_pool(name="qk", bufs=1))
        v_pool = ctx.enter_context(tc.tile_pool(name="vp", bufs=1))
        x_pool = ctx.enter_context(tc.tile_pool(name="xp", bufs=24))
        e_pool = ctx.enter_context(tc.tile_pool(name="ep", bufs=4))
        oc_pool = ctx.enter_context(tc.tile_pool(name="oc", bufs=6))
        small = ctx.enter_context(tc.tile_pool(name="small", bufs=2))
        outst_pool = ctx.enter_context(tc.tile_pool(name="outst", bufs=2))
        scr_pool = ctx.enter_context(tc.tile_pool(name="scr", bufs=2,
                                                  space="DRAM"))
        # PSUM: sp = score tiles + pass-1 projection interludes (2x2 banks),
        # acc = u accumulators / boundary bursts (4x1 bank)
        sp = ctx.enter_context(tc.tile_pool(name="sp", bufs=2, space="PSUM"))
        acc = ctx.enter_context(tc.tile_pool(name="acc", bufs=4,
                                             space="PSUM"))

        bqk_sb = consts.tile([128, MT], f32)
        maskb_sb = consts.tile([128, JT], f32)
        # reciprocal-broadcast staging: the four softmax-denominator vectors
        # are DMAed to partitions 0/32/64/96 (the only legal stationary base
        # partitions), reciprocated in one DVE op, then broadcast across 64
        # partitions by a K=1 ones-matmul -- no DRAM round trip
        dng_a = consts.tile([65, 512], f32, name="dnga", tag="dnga")
        dng_b = consts.tile([1, 512], f32, name="dngb", tag="dngb")
        rg_a = consts.tile([65, 512], f32, name="rga", tag="rga")
        rg_b = consts.tile([1, 512], f32, name="rgb", tag="rgb")
        rgr_a = consts.tile([65, 512], f32r, name="rgra", tag="rgra")
        rgr_b = consts.tile([1, 512], f32r, name="rgrb", tag="rgrb")
        ones_bc = consts.tile([65, 64], f32r, name="onesbc", tag="onesbc")
        nc.vector.memset(dng_a.bitcast(mybir.dt.uint32), 0)
        nc.vector.memset(ones_bc.bitcast(mybir.dt.float32), 1.0)
        # dn vector i lives at (tile, base partition): 0/32/64 in dng_a,
        # the fourth in dng_b at base 0 (legal bases are only 0/32/64)
        dn_slot = [(dng_a, rgr_a, 0), (dng_a, rgr_a, 32),
                   (dng_a, rgr_a, 64), (dng_b, rgr_b, 0)]
        wo_sb = [consts.tile([64, D], f32r, name=f"wo{i}", tag=f"wo{i}")
                 for i in range(HG)]

        wqk_sb = [w_pool.tile([128, D], bf16, name=f"wqk{k}", tag=f"wqk{k}")
                  for k in range(KT)]
        wv_sb = [w_pool.tile([128, HG * DH], bf16, name=f"wv{k}",
                             tag=f"wv{k}") for k in range(KT)]

        # q pair tiles: q_t[m][p], heads 2p (rows 0:64) and 2p+1 (rows 64:128)
        q_t = [[qk_pool.tile([128, S], bf16, name=f"q{m}p{p}",
                             tag=f"q{m}p{p}") for p in range(2)]
               for m in range(2)]
        # zero-padded k tiles: kz[m][hl] has k rows in parity half, 0 in other
        kz = [[qk_pool.tile([128, S], bf16, name=f"kz{m}h{hl}",
                            tag=f"kz{m}h{hl}") for hl in range(HG)]
              for m in range(2)]
        # v in [keys, jt, hl, DH+1] layout; column DH holds ones
        v_sb = v_pool.tile([128, JT, HG, DH + 1], f32r)

        # ---------------- DMA prefetch (priority order) ----------------
        def load_x(c, ks=range(KT)):
            xs = []
            for k in ks:
                x_one = x_pool.tile([128, 512], bf16, name="xtc", tag="xtc")
                nc.sync.dma_start(
                    out=x_one,
                    in_=xt_d[k * 128:(k + 1) * 128, c * 512:(c + 1) * 512])
                xs.append(x_one)
            return xs

        # interleave wqk / x(c0) per k-tile so the head's k-outer rounds can
        # start on the first pair instead of waiting for all of wqk
        x_c = {0: []}
        for k in range(KT):
            nc.sync.dma_start(out=wqk_sb[k],
                              in_=wqk_d[k * 128:(k + 1) * 128, :])
            x_c[0] += load_x(0, [k])
        nc.sync.dma_start(out=bqk_sb, in_=bqk_d)
        nc.sync.dma_start(out=maskb_sb, in_=maskb_d)
        for k in range(KT):
            nc.sync.dma_start(out=wv_sb[k],
                              in_=wv_d[k * 128:(k + 1) * 128, :])
        # kz zero halves + v ones column on the (idle) engines
        for m in range(2):
            for hl in range(HG):
                zh = 1 - (hl % 2)
                # memset lacks a 16-bit value type; zero through a u32 view
                nc.vector.memset(
                    kz[m][hl][zh * 64:(zh + 1) * 64, :].bitcast(
                        mybir.dt.uint32), 0)
        nc.vector.memset(v_sb[:, :, :, DH:DH + 1].bitcast(mybir.dt.float32),
                         1.0)
        x_c[1] = load_x(1)
        x_c[2] = load_x(2)
        for i in range(HG):
            nc.sync.dma_start(out=wo_sb[i], in_=wo_d[i * 64:(i + 1) * 64, :])
        x_c[3] = load_x(3)

        # ---------------- projection building blocks ----------------
        def emit_round(c, mts, pps, xtc, korder=True):
            """Accumulate wqk col-blocks `mts` for chunk c into the given
            psum APs, then bias-add/cast into q_t / kz (bf16). korder=True
            interleaves the k-loop across accumulators (requires separate
            psum tiles); korder=False keeps each accumulator's group
            contiguous (safe for slices of a shared psum tile)."""
            nsl = slice(c * 512, (c + 1) * 512)
            if korder:
                for k in range(KT):
                    for i, mt in enumerate(mts):
                        nc.tensor.matmul(
                            pps[i],
                            wqk_sb[k][:, mt * 128:(mt + 1) * 128],
                            xtc[k],
                            start=(k == 0), stop=(k == KT - 1))
            else:
                for i, mt in enumerate(mts):
                    for k in range(KT):
                        nc.tensor.matmul(
                            pps[i],
                            wqk_sb[k][:, mt * 128:(mt + 1) * 128],
                            xtc[k],
                            start=(k == 0), stop=(k == KT - 1))
            for i, mt in enumerate(mts):
                pp = pps[i]
                if mt < 4:
                    m, p = mt // 2, mt % 2
                    nc.vector.tensor_scalar_add(q_t[m][p][:, nsl], pp,
                                                bqk_sb[:, mt:mt + 1])
                else:
                    m, pr = (mt - 4) // 2, (mt - 4) % 2
                    for eps in range(2):
                        hl = 2 * pr + eps
                        esl = slice(eps * 64, (eps + 1) * 64)
                        nc.vector.tensor_scalar_add(
                            kz[m][hl][esl, nsl], pp[esl, :],
                            bqk_sb[esl, mt:mt + 1])

        def qk_round_acc(c, mts):
            pps = [acc.tile([128, 512], f32, name="accp", tag="acc")
                   for _ in mts]
            emit_round(c, mts, pps, x_c[c])

        def v_round(c, pps, xtc, korder=True):
            """v projection of chunk c: pps = 4 psum APs [128, 256]."""
            if korder:
                for k in range(KT):
                    for sl in range(4):
                        nc.tensor.matmul(
                            pps[sl],
                            xtc[k][:, sl * 128:(sl + 1) * 128],
                            wv_sb[k],
                            start=(k == 0), stop=(k == KT - 1))
            else:
                for sl in range(4):
                    for k in range(KT):
                        nc.tensor.matmul(
                            pps[sl],
                            xtc[k][:, sl * 128:(sl + 1) * 128],
                            wv_sb[k],
                            start=(k == 0), stop=(k == KT - 1))
            for sl in range(4):
                st = c * 4 + sl
                nc.vector.tensor_copy(
                    out=v_sb[:, st, :, 0:DH],
                    in_=pps[sl].rearrange("p (h d) -> p h d", h=HG))

        def v_round_acc(c):
            pps = [acc.tile([128, 256], f32, name="accv", tag="acc")
                   for _ in range(4)]
            v_round(c, pps, x_c[c])

        # interlude "pieces": single sp-pool allocations (~3.4us of PE work)
        # slotted between attention iterations so projections and the
        # out-projection never need their own PSUM banks mid-pass
        def qk_piece(c, mts):
            a = sp.tile([128, 1024], f32, name="pp", tag="sp")
            emit_round(c, mts, [a[:, 0:512], a[:, 512:1024]], x_c[c],
                       korder=False)

        def v_piece(c):
            v = sp.tile([128, 1024], f32, name="vi", tag="sp")
            v_round(c, [v[:, i * 256:(i + 1) * 256] for i in range(4)],
                    x_c[c], korder=False)

        # ---------------- attention pass ----------------
        oc_store = {}

        def attention_pass(c, p, interludes):
            csl = slice(c * 512, (c + 1) * 512)
            # u tiles are allocated after the first two score groups so the
            # wait on the previous combine's psum reads overlaps j0/j1
            u_tiles = []

            def alloc_u():
                for name in ("u1a", "u1b", "u2a", "u2b"):
                    u_tiles.append(acc.tile([DH + 1, 512], f32, name=name,
                                            tag="acc"))

            def make_pv(j, e_m):
                # eps-outer so consecutive matmuls share the stationary v
                def pv():
                    for eps in range(2):
                        for mi in range(2):
                            nc.tensor.matmul(
                                u_tiles[2 * mi + eps],
                                v_sb[:, j, 2 * p + eps, :],
                                e_m[mi][:, eps * 512:(eps + 1) * 512],
                                start=(j == 0), stop=(j == JT - 1))
                return pv

            # PV lags the scores by one iteration so the PE never waits on
            # the exp: per j it runs scores(j), PV(j-1), then any interlude
            # piece (whose first op waits on this j's first exp anyway)
            prev_pv = None
            for j in range(JT):
                jsl = slice(j * 128, (j + 1) * 128)
                e_m = []
                for m in range(2):
                    s_ps = sp.tile([128, 1024], f32, name=f"s{m}", tag="sp")
                    for eps in range(2):
                        nc.tensor.matmul(
                            s_ps[:, eps * 512:(eps + 1) * 512],
                            kz[m][2 * p + eps][:, jsl],
                            q_t[m][p][:, csl],
                            start=True, stop=True)
                    e_sb = e_pool.tile([128, 1024], f32r, name=f"e{m}",
                                       tag=f"e{m}")
                    nc.scalar.activation(e_sb, s_ps, Exp,
                                         bias=maskb_sb[:, j:j + 1],
                                         scale=SCALE)
                    e_m.append(e_sb)
                if j == 1:
                    alloc_u()
                if prev_pv is not None:
                    prev_pv()
                for piece in interludes.get(j, ()):
                    piece()
                prev_pv = make_pv(j, e_m)
            prev_pv()
            # combine front: o_hl = u1/dn1 - lam*u2/dn2. Stage u (keeping
            # row 64 = denominator, freeing each psum bank after one DVE
            # op), gather the 4 dn rows to stationary-legal partitions, and
            # reciprocate. The back half (broadcast matmuls + multiplies)
            # is deferred to a piece slot in the NEXT pass so its PE ops
            # never stall the in-order queue on the reciprocal.
            u_sbs = []
            for eps in range(2):
                u1_sb = small.tile([DH + 1, 512], f32, name="u1_sb",
                                   tag="u1_sb")
                nc.vector.tensor_copy(out=u1_sb, in_=u_tiles[0 + eps])
                u2_sb = small.tile([DH + 1, 512], f32, name="u2_sb",
                                   tag="u2_sb")
                nc.vector.tensor_copy(out=u2_sb, in_=u_tiles[2 + eps])
                u_sbs.append((u1_sb, u2_sb))
                d1, _, r1 = dn_slot[2 * eps]
                d2, _, r2 = dn_slot[2 * eps + 1]
                nc.sync.dma_start(out=d1[r1:r1 + 1, :],
                                  in_=u1_sb[DH:DH + 1, :])
                nc.sync.dma_start(out=d2[r2:r2 + 1, :],
                                  in_=u2_sb[DH:DH + 1, :])
            nc.vector.reciprocal_approx_fast(out=rg_a, in_=dng_a)
            nc.vector.reciprocal_approx_fast(out=rg_b, in_=dng_b)
            # f32r operands must be explicitly rounded; copy does that
            nc.vector.tensor_copy(out=rgr_a, in_=rg_a)
            nc.vector.tensor_copy(out=rgr_b, in_=rg_b)

            def combine_back():
                ochl = oc_store.setdefault(c, [None] * HG)
                for eps in range(2):
                    bcp = sp.tile([64, 2, 512], f32, name="bcp", tag="sp")
                    for jj in range(2):
                        _, rt, i = dn_slot[2 * eps + jj]
                        nc.tensor.matmul(bcp[:, jj, :],
                                         ones_bc[i:i + 1, :],
                                         rt[i:i + 1, :],
                                         start=True, stop=True)
                    hl = 2 * p + eps
                    u1_sb, u2_sb = u_sbs[eps]
                    t1 = small.tile([64, 512], f32, name="t1", tag="t1")
                    nc.vector.tensor_tensor(t1, u1_sb[0:DH, :],
                                            bcp[:, 0, :], mult)
                    t2 = small.tile([64, 512], f32, name="t2", tag="t2")
                    nc.vector.tensor_tensor(t2, u2_sb[0:DH, :],
                                            bcp[:, 1, :], mult)
                    oc_t = oc_pool.tile([64, 512], f32r, name="oc_t",
                                        tag="oc")
                    nc.vector.scalar_tensor_tensor(
                        out=oc_t, in0=t2, scalar=-float(lam), in1=t1,
                        op0=mult, op1=add)
                    ochl[hl] = oc_t
            return combine_back

        def outproj_mts(c, mts, pps):
            csl_o = slice(c * 512, (c + 1) * 512)
            ochl_o = oc_store[c]
            for i, mt in enumerate(mts):
                op = pps[i]
                for hl in range(HG):
                    nc.tensor.matmul(op,
                                     wo_sb[hl][:, mt * 128:(mt + 1) * 128],
                                     ochl_o[hl],
                                     start=(hl == 0), stop=(hl == HG - 1))
                outst = outst_pool.tile([128, 512], f32, name="outst",
                                        tag="outst")
                nc.vector.tensor_copy(out=outst, in_=op)
                nc.sync.dma_start(out=out_d[mt * 128:(mt + 1) * 128, csl_o],
                                  in_=outst)

        def op_piece(c, half):
            t = sp.tile([128, 1024], f32, name="op", tag="sp")
            outproj_mts(c, (2 * half, 2 * half + 1),
                        [t[:, 0:512], t[:, 512:1024]])

        def outproj_acc(c):
            for mt in range(MT):
                op = acc.tile([128, 512], f32, name="accop", tag="acc")
                outproj_mts(c, (mt,), [op])

        # ---------------- schedule ----------------
        # head: only what pass (0,0) needs before its first scores -- the
        # pair-0 k and q projections of chunk 0; everything else (v(0),
        # pair-1 chunk-0, other chunks) follows as in-pass pieces
        qk_round_acc(0, (4, 6))
        qk_round_acc(0, (0, 2))

        # Interlude pieces per pass, keyed by the attention iteration they
        # follow. Deadlines: key tile j reads kz/v of chunk j//4, so chunk
        # c's k/v pieces must precede j = 4c. Out-proj of chunk c runs 1.5
        # passes after its combine so it never waits on the combine's
        # DVE/DMA tail.
        P = lambda f, *a: (lambda: f(*a))
        # pass (0,0) piece order: chunk c's k pieces before j=4c, its v piece
        # by j=4c (PV lags one iteration); all x0 readers (chunk-0 pair-1
        # q/k) finish by j7 so the x3 loads can recycle their buffers before
        # the chunk-3 pieces at j9 need them (in-order PE queue)
        pieces = {
            (0, 0): {0: [P(v_piece, 0)],
                     1: [P(qk_piece, 1, (4, 5))],
                     2: [P(qk_piece, 1, (6, 7))],
                     3: [P(qk_piece, 0, (5, 7))],
                     4: [P(v_piece, 1)],
                     5: [P(qk_piece, 2, (4, 5))],
                     6: [P(qk_piece, 2, (6, 7))],
                     7: [P(qk_piece, 0, (1, 3))],
                     8: [P(v_piece, 2)],
                     9: [P(qk_piece, 3, (4, 5))],
                     10: [P(qk_piece, 3, (6, 7))],
                     11: [P(v_piece, 3)]},
            (0, 1): {0: [P(qk_piece, 1, (0, 1))],
                     2: [P(qk_piece, 1, (2, 3))]},
            (1, 0): {0: [P(qk_piece, 2, (0, 1))],
                     2: [P(qk_piece, 2, (2, 3))]},
            (1, 1): {0: [P(op_piece, 0, 0)], 2: [P(op_piece, 0, 1)],
                     4: [P(op_piece, 0, 2)], 6: [P(op_piece, 0, 3)]},
            (2, 0): {0: [P(qk_piece, 3, (0, 1))],
                     2: [P(qk_piece, 3, (2, 3))]},
            (2, 1): {0: [P(op_piece, 1, 0)], 2: [P(op_piece, 1, 1)],
                     4: [P(op_piece, 1, 2)], 6: [P(op_piece, 1, 3)]},
            (3, 1): {0: [P(op_piece, 2, 0)], 2: [P(op_piece, 2, 1)],
                     4: [P(op_piece, 2, 2)], 6: [P(op_piece, 2, 3)]},
        }
        prev_back = None
        for c in range(NCH):
            for p in range(2):
                pm = dict(pieces.get((c, p), {}))
                if prev_back is not None:
                    # previous pass's combine back-half at j1: its inputs
                    # (reciprocal) are long done, so no PE stall
                    pm[1] = [prev_back] + list(pm.get(1, ()))
                prev_back = attention_pass(c, p, pm)

        # tail: keep the PE warm through the last combine's latency chain so
        # the HAM governor doesn't halve the clock under the final out-proj
        def warm_mms(n):
            w = sp.tile([128, 1024], f32, name="warm", tag="sp")
            for r in range(n):
                nc.tensor.matmul(w[:, (r % 2) * 512:(r % 2) * 512 + 512],
                                 kz[0][0][:, 0:128], q_t[0][0][:, 0:512],
                                 start=True, stop=True,
                                 skip_group_check=True)
        warm_mms(20)
        prev_back()
        warm_mms(12)
        outproj_acc(NCH - 1)

    nc.compile()
    return nc


def _get_nc(lam: float):
    key = round(float(lam), 8)
    if key not in _BUILD_CACHE:
        _BUILD_CACHE[key] = _build(float(lam))
    return _BUILD_CACHE[key]


def _prep_in_maps(hidden_states, attention_mask, Wq, bq, Wk, bk, Wv, bv, Wo,
                  lam_f):
    import ml_dtypes
    bf16 = ml_dtypes.bfloat16
    in_maps = []
    for d in range(NCORES):
        b, g = d // 4, d % 4
        gc = slice(g * HG * DH, (g + 1) * HG * DH)   # 256 head-group columns
        xt = np.ascontiguousarray(hidden_states[b].T).astype(bf16)
        wqk = np.ascontiguousarray(
            np.concatenate([Wq[:, :D][:, gc], Wq[:, D:][:, gc],
                            Wk[:, :D][:, gc], Wk[:, D:][:, gc]],
                           axis=1)).astype(bf16)
        wv = np.ascontiguousarray(Wv[:, gc]).astype(bf16)
        wo = np.ascontiguousarray(Wo[gc, :])
        bqk = np.ascontiguousarray(
            np.concatenate([bq[:D][gc], bq[D:][gc], bk[:D][gc], bk[D:][gc]])
            .reshape(MT, 128).T)
        maskb = np.ascontiguousarray(
            ((1.0 - attention_mask[b]) * -10000.0).reshape(JT, 128).T)
        in_maps.append({"xt": xt, "wqk": wqk, "wv": wv, "wo": wo,
                        "bqk": bqk, "maskb": maskb})
    return in_maps


def kernel(hidden_states, attention_mask, Wq, bq, Wk, bk, Wv, bv, Wo, bo,
           lam):
    hidden_states = np.asarray(hidden_states, dtype=np.float32)
    attention_mask = np.asarray(attention_mask, dtype=np.float32)
    Wq = np.asarray(Wq, dtype=np.float32)
    bq = np.asarray(bq, dtype=np.float32)
    Wk = np.asarray(Wk, dtype=np.float32)
    bk = np.asarray(bk, dtype=np.float32)
    Wv = np.asarray(Wv, dtype=np.float32)
    bv = np.asarray(bv, dtype=np.float32)
    Wo = np.asarray(Wo, dtype=np.float32)
    bo = np.asarray(bo, dtype=np.float32)
    lam_f = float(np.asarray(lam))

    from concourse.bass_utils import run_bass_kernel_spmd

    nc = _get_nc(lam_f)
    in_maps = _prep_in_maps(hidden_states, attention_mask, Wq, bq, Wk, bk,
                            Wv, bv, Wo, lam_f)
    res = run_bass_kernel_spmd(nc, in_maps, core_ids=list(range(NCORES)))

    out = np.zeros((B, S, D), np.float32)
    for d in range(NCORES):
        out[d // 4] += res.results[d]["outT"].T
    out += bo
    # v-bias correction is linear: o += (1-lam)*bv @ Wo (exact; bv is zero in
    # the reference setup, so this is a no-op there)
    if np.any(bv != 0.0):
        out += ((1.0 - lam_f) * bv) @ Wo
    return out
